# revision 14
# baseline (speedup 1.0000x reference)
"""Trainium2 Bass kernel for nn_Attention_Emb (dense transformer attention
with embedding-selected QKV projections and a relative-position branch).

Sharding: 16 (batch, head) units, 2 per core across 8 NeuronCores.

Math notes (exact reductions, no approximation beyond bf16 matmul inputs):
- pos_attn[b,h,s,t] = softmax_t((ph[s]-ph[t])@hw + hb) = softmax_t(-ph[t]@hw)
  is independent of s (shift invariance) -> a single row p[t] per (b,h);
  its contribution to the output is the rank-1 term p @ v.
- softmax over t of ((k0[t]+s)@(q0[s]+s)) == softmax over t of (k0[t]@(q0[s]+s))
  (terms constant in t cancel), so the strength bias is only applied to q.
- v = v0 + s with sum_t attn = 1 -> the +s contribution is a constant bias.
- final renormalization divides by sum((1-g)*A + g*P) == 1 exactly.
- the output projection is fused into the v projection:
  VW[t,m] = sum_d v0[t,d]*OW[d,m]  via rhs = blkdiag((out_w @ ve).T).

Device computes, per unit, in a t-on-partitions layout:
  Qs = (blkdiag(qe.T) @ xu + s)/sqrt(128), K0 = blkdiag(ke.T) @ xu,
  VW[t,m] (t on partitions), E[t,s] = exp(K0[:,t]@Qs), Z[s] = ones@E,
  M1[m,s] = sum_t VW[t,m] E[t,s], pvw[m] = sum_t VW[t,m] p[t].
Host combines: out = (1-g)/Z * M1 + g*pvw + (blkdiag(out_w.T).T@s + out_b).
"""

import numpy as np
import ml_dtypes

BF16 = ml_dtypes.bfloat16

B, S, W, DIM, HEADS = 4, 1024, 8, 64, 4
HD = 128
NCORES = 8
SQ = (slice(0, 512), slice(512, 1024))

_prog_cache = {}


def _split_multiwait_insts(nc):
    """walrus codegen rejects instructions carrying >1-2 sem waits; move the
    extras onto preceding same-engine NoOps (equivalent: engine executes its
    instructions in program order)."""
    import concourse.mybir as mybir

    for f in nc.m.functions:
        for bb in f.blocks:
            insts = bb.instructions
            i = 0
            while i < len(insts):
                inst = insts[i]
                si = inst.sync_info
                cap = 2 if type(inst).__name__ == "InstEventSemaphoreOp" else 1
                if si is not None and len(si.on_wait) > cap:
                    waits = list(si.on_wait)
                    extra, keep = waits[:-cap], waits[-cap:]
                    new = []
                    for k, w in enumerate(extra):
                        nop = mybir.InstNoOp(
                            name=f"{inst.name}_splitw{k}", ins=[], outs=[]
                        )
                        nop.engine = inst.engine
                        nop.sync_info = mybir.SyncInfo(on_wait=[w], on_update=[])
                        new.append(nop)
                    inst.sync_info = mybir.SyncInfo(
                        on_wait=keep, on_update=list(si.on_update)
                    )
                    insts[i:i] = new
                    i += len(new)
                i += 1


def _build_program():
    if "nc" in _prog_cache:
        return _prog_cache["nc"]
    import concourse.bass as bass
    import concourse.mybir as mybir
    import concourse.tile as tile

    f32 = mybir.dt.float32
    bf16 = mybir.dt.bfloat16
    AF = mybir.ActivationFunctionType
    ts = bass.ts

    nc = bass.Bass(trn_type="TRN2")
    xu = nc.dram_tensor("xu", [2, 128, S], bf16, kind="ExternalInput")
    lq = nc.dram_tensor("lq", [128, 128], bf16, kind="ExternalInput")
    lk = nc.dram_tensor("lk", [128, 128], bf16, kind="ExternalInput")
    rvw = nc.dram_tensor("rvw", [128, 128], bf16, kind="ExternalInput")
    ones_in = nc.dram_tensor("ones", [128, 1], bf16, kind="ExternalInput")
    bq = nc.dram_tensor("bq", [128, 1], f32, kind="ExternalInput")
    pc = nc.dram_tensor("pc", [2, 128, 16], bf16, kind="ExternalInput")
    m1o = nc.dram_tensor("m1o", [2, 128, S + 1], bf16, kind="ExternalOutput")
    zo = nc.dram_tensor("zo", [2, 1, S], f32, kind="ExternalOutput")

    def _light_drain_and_barrier(self, tick_clock, wait_clock):
        from concourse.vector_clock import ScopedClock

        drain_inst = self.nc.sync.drain()
        wait_clock.add_sem_waits(
            drain_inst.ins, ScopedClock({None: tick_clock.global_clock})
        )
        self.nc.all_engine_barrier()
        popped = self.nc._tile_sem_poison_stack.pop()
        assert popped is self._sem_poison
        self.nc.clear_and_free_semaphores(list(self.sems.allocated().values()))

    orig_dab = tile.TileContext._drain_and_barrier
    tile.TileContext._drain_and_barrier = _light_drain_and_barrier

    with tile.TileContext(nc) as tc:
        with (
            tc.tile_pool(name="wp", bufs=1) as wp,
            tc.tile_pool(name="xp", bufs=1) as xp,
            tc.tile_pool(name="sp", bufs=1) as sp,
            tc.tile_pool(name="op", bufs=1) as op,
            tc.tile_pool(name="pa", bufs=2, space="PSUM") as pa,
            tc.tile_pool(name="pb", bufs=1, space="PSUM") as pb,
        ):
            # PE warm-up: dummy matmuls with no input deps keep the PE
            # busy through the DMA-wait window and trip the HAM un-throttle
            # before real work starts.
            WM1 = wp.tile([128, 1], bf16, name="WM1")
            nc.vector.memset(WM1, 1.0)
            WM2 = wp.tile([128, 512], bf16, name="WM2")
            nc.vector.memset(WM2, 0.0)
            WME = wp.tile([1, 2], f32, name="WME")
            nc.vector.memset(WME, 0.0)
            nc.scalar.activation(WME, WME, AF.Exp)
            with tc.tile_pool(name="pw", bufs=1, space="PSUM") as pw:
                WPP = pw.tile([1, 512], f32, name="WPP")
                for _ in range(9):
                    nc.tensor.matmul(WPP, WM1, WM2, start=True, stop=True)

            # inputs: unit0 chunks on sync queue, unit1 on scalar queue
            XUs = []
            for j in range(2):
                XU = xp.tile([128, S], bf16, name=f"xus{j}")
                eng = nc.sync if j == 0 else nc.scalar
                for q in range(2):
                    eng.dma_start(out=XU[:, SQ[q]], in_=xu[j][:, SQ[q]])
                XUs.append(XU)

            # small constant loads on the scalar-engine HWDGE queue
            LQ = wp.tile([128, 128], bf16, name="LQ")
            nc.scalar.dma_start(out=LQ, in_=lq[:, :])
            LK = wp.tile([128, 128], bf16, name="LK")
            nc.scalar.dma_start(out=LK, in_=lk[:, :])
            RVW = wp.tile([128, 128], bf16, name="RVW")
            nc.scalar.dma_start(out=RVW, in_=rvw[:, :])
            ONES = wp.tile([128, 1], bf16, name="ONES")
            nc.scalar.dma_start(out=ONES, in_=ones_in[:, :])
            BQ = wp.tile([128, 1], f32, name="BQ")
            nc.scalar.dma_start(out=BQ, in_=bq[:, :])
            PCs = []
            for j in range(2):
                PCj = wp.tile([128, 16], bf16, name=f"PC{j}")
                nc.scalar.dma_start(out=PCj, in_=pc[j])
                PCs.append(PCj)

            QSs, KSs, VWSs, ETss, E2ss, UPs = [], [], [], [], [], []
            # ---- Q/K projections for BOTH units first (QS/KS gate scores)
            for j in range(2):
                XU = XUs[j]
                QP = pa.tile([128, S], f32, name=f"qp{j}", tag="pa")
                for q in range(2):
                    nc.tensor.matmul(
                        QP[:, SQ[q]], LQ, XU[:, SQ[q]], start=True, stop=True
                    )
                QS = sp.tile([128, S], bf16, name=f"qs{j}")
                nc.scalar.activation(
                    QS, QP, AF.Identity, bias=BQ[:, 0:1], scale=float(1.0 / np.sqrt(HD))
                )
                QSs.append(QS)
                KP = pa.tile([128, S], f32, name=f"kp{j}", tag="pa")
                for q in range(2):
                    nc.tensor.matmul(
                        KP[:, SQ[q]], LK, XU[:, SQ[q]], start=True, stop=True
                    )
                KS = sp.tile([128, S], bf16, name=f"ks{j}")
                nc.vector.tensor_copy(KS, KP)
                KSs.append(KS)

            def emit_vw(j):
                VWS = sp.tile([128, S], bf16, name=f"vws{j}")
                VP = pa.tile([128, S], f32, name=f"vp{j}", tag="pa")
                for c in range(8):
                    nc.tensor.matmul(
                        VP[:, ts(c, 128)],
                        XUs[j][:, ts(c, 128)],
                        RVW,
                        start=True,
                        stop=True,
                    )
                nc.vector.tensor_copy(VWS, VP)
                VWSs.append(VWS)

            def emit_scores_exp(j):
                ETs = []
                for c in range(8):
                    SP_ = pa.tile([128, S], f32, name=f"sp{j}_{c}", tag="pa")
                    for q in range(2):
                        nc.tensor.matmul(
                            SP_[:, SQ[q]],
                            KSs[j][:, ts(c, 128)],
                            QSs[j][:, SQ[q]],
                            start=True,
                            stop=True,
                        )
                    ET = sp.tile([128, S], bf16, name=f"et{j}_{c}")
                    nc.scalar.activation(ET, SP_, AF.Exp)
                    ETs.append(ET)
                ETss.append(ETs)
                E2s = []
                for i in range(4):
                    E2 = sp.tile([128, S], bf16, name=f"e2_{j}_{i}")
                    nc.vector.tensor_add(E2, ETs[2 * i], ETs[2 * i + 1])
                    E2s.append(E2)
                E2ss.append(E2s)

            # vw0 first (its cast must precede AV0), then unit0 scores+exp,
            # vw1 + unit1 scores under the exp0 phase
            emit_vw(0)
            emit_scores_exp(0)
            emit_vw(1)
            emit_scores_exp(1)

            # ---- AV (M1 + pvw), then Z, per unit; exp-paced
            for j in range(2):
                ETs = ETss[j]
                UP = pb.tile([128, 1536], f32, name=f"up{j}", tag="pb")
                for c in range(8):
                    st, spf = (c == 0), (c == 7)
                    for q in range(2):
                        nc.tensor.matmul(
                            UP[:, SQ[q]],
                            VWSs[j][:, ts(c, 128)],
                            ETs[c][:, SQ[q]],
                            start=st,
                            stop=spf,
                        )
                    nc.tensor.matmul(
                        UP[:, 1024:1026],
                        VWSs[j][:, ts(c, 128)],
                        PCs[j][:, 2 * c : 2 * c + 2],
                        start=st,
                        stop=spf,
                    )
                UPs.append(UP)
                ZP = pa.tile([1, S], f32, name=f"zp{j}", tag="pa")
                for i in range(4):
                    for q in range(2):
                        nc.tensor.matmul(
                            ZP[0:1, SQ[q]],
                            ONES,
                            E2ss[j][i][:, SQ[q]],
                            start=(i == 0),
                            stop=(i == 3),
                        )
                ZS = op.tile([1, S], f32, name=f"zs{j}")
                nc.vector.tensor_copy(ZS, ZP)
                nc.sync.dma_start(out=zo[j], in_=ZS)

            # ---- copy + store M1 (with pvw as col 1024) per chunk, two queues
            for j in range(2):
                MS = op.tile([128, S + 1], bf16, name=f"ms{j}")
                nc.vector.tensor_copy(MS[:, SQ[0]], UPs[j][:, SQ[0]])
                nc.sync.dma_start(out=m1o[j][:, SQ[0]], in_=MS[:, SQ[0]])
                nc.scalar.activation(MS[:, SQ[1]], UPs[j][:, SQ[1]], AF.Copy)
                nc.vector.tensor_copy(MS[:, 1024:1025], UPs[j][:, 1024:1025])
                nc.scalar.dma_start(out=m1o[j][:, 512:1025], in_=MS[:, 512:1025])

    tile.TileContext._drain_and_barrier = orig_dab
    _split_multiwait_insts(nc)
    _prog_cache["nc"] = nc
    return nc


def _blkdiag(m):
    z = np.zeros((64, 64), np.float32)
    return np.block([[m, z], [z, m]]).astype(np.float32)


def _prep(inputs):
    f32 = np.float32
    x = np.asarray(inputs["x"], f32)
    pos = np.asarray(inputs["pos"], f32)
    strength = np.asarray(inputs["strength"], f32)
    eid = int(np.asarray(inputs["embed_id1"]))
    qe = np.asarray(inputs["q_emb_w"], f32)[eid].reshape(DIM, DIM)
    ke = np.asarray(inputs["k_emb_w"], f32)[eid].reshape(DIM, DIM)
    ve = np.asarray(inputs["v_emb_w"], f32)[eid].reshape(DIM, DIM)
    pos_w1 = np.asarray(inputs["pos_w1"], f32)
    pos_b1 = np.asarray(inputs["pos_b1"], f32)
    pos_w2 = np.asarray(inputs["pos_w2"], f32)
    pos_b2 = np.asarray(inputs["pos_b2"], f32)
    head_w = np.asarray(inputs["head_w"], f32)
    gate = np.asarray(inputs["gate"], f32)
    out_w = np.asarray(inputs["out_w"], f32)
    out_b = np.asarray(inputs["out_b"], f32)
    str_w = np.asarray(inputs["str_w"], f32)
    str_b = np.asarray(inputs["str_b"], f32)

    s_vec = (strength @ str_w.T + str_b).astype(f32)
    Lq = _blkdiag(np.ascontiguousarray(qe.T))
    Lk = _blkdiag(np.ascontiguousarray(ke.T))
    Rvw = _blkdiag(np.ascontiguousarray((out_w @ ve).T))
    Low = _blkdiag(np.ascontiguousarray(out_w.T))
    BQ = (np.tile(s_vec, 2) / np.sqrt(HD)).astype(f32).reshape(128, 1)
    ones_arr = np.ones((128, 1), f32)

    # relative-position branch: softmax_t((ph[s]-ph[t])@hw + hb) = softmax_t(-ph[t]@hw)
    t1 = np.maximum(pos @ pos_w1.T + pos_b1, 0.0).astype(f32)
    ph = (t1 @ pos_w2.T + pos_b2).astype(f32)  # [B, S, 8]
    a = np.einsum("btd,hd->bht", ph, head_w).astype(f32)  # [B, H, S]
    na = -a
    na = na - na.max(axis=-1, keepdims=True)
    e = np.exp(na)
    pvec = (e / e.sum(axis=-1, keepdims=True)).astype(f32)  # [B, H, S]

    g = (1.0 / (1.0 + np.exp(-gate))).astype(f32)  # [H]

    in_maps = []
    for core in range(NCORES):
        xuarr = np.empty((2, 128, S), f32)
        pcarr = np.zeros((2, 128, 16), f32)
        for j in range(2):
            u = 2 * core + j
            b, h = divmod(u, HEADS)
            xuarr[j] = x[b, :, :, 2 * h : 2 * h + 2].transpose(2, 0, 1).reshape(128, S)
            pcarr[j, :, 0::2] = pvec[b, h].reshape(8, 128).T
        in_maps.append(
            dict(
                xu=np.ascontiguousarray(xuarr).astype(BF16),
                lq=Lq.astype(BF16),
                lk=Lk.astype(BF16),
                rvw=Rvw.astype(BF16),
                ones=ones_arr.astype(BF16),
                bq=BQ,
                pc=np.ascontiguousarray(pcarr).astype(BF16),
            )
        )
    meta = dict(g=g, s_vec=s_vec, Low=Low, out_b=out_b)
    return in_maps, meta


def _post(results, meta):
    f32 = np.float32
    g = meta["g"]
    s_tiled = np.tile(meta["s_vec"], 2).astype(f32)  # [128]
    outb_tiled = np.tile(meta["out_b"], 2).astype(f32)  # [128]
    cb0 = meta["Low"].T @ s_tiled + outb_tiled  # [128]
    out = np.empty((B, S, W, DIM), f32)
    for core in range(NCORES):
        r = results[core]
        for j in range(2):
            u = 2 * core + j
            b, h = divmod(u, HEADS)
            M1full = np.asarray(r["m1o"][j], f32)  # [128, S+1]
            M1 = M1full[:, :S]
            Z = r["zo"][j][0]  # [S]
            pvw = M1full[:, S]  # [128]
            cb = g[h] * pvw + cb0  # [128]
            F = (1.0 - g[h]) * M1 / Z[None, :] + cb[:, None]
            out[b, :, 2 * h : 2 * h + 2, :] = F.reshape(2, DIM, S).transpose(2, 0, 1)
    return out


def kernel(**inputs) -> np.ndarray:
    import time

    from concourse.bass_utils import run_bass_kernel_spmd

    nc = _build_program()
    in_maps, meta = _prep(inputs)
    try:
        res = run_bass_kernel_spmd(nc, in_maps, core_ids=list(range(NCORES)))
    except Exception:
        # one retry: a previous process can leave a core wedged transiently
        time.sleep(3.0)
        res = run_bass_kernel_spmd(nc, in_maps, core_ids=list(range(NCORES)))
    return _post(res.results, meta)
